# revision 31
# baseline (speedup 1.0000x reference)
"""Trainium2 Bass kernel for nn_MultiHeadedAttention_6416681140387.

Two-branch windowed video attention:
  x [8,256,96,96] -> 1x1 conv Q/K/V -> per-branch full attention over
  window-token features (branch0: 4x4 patches, d=2048, 2304 key tokens;
  branch1: 8x8 patches, d=8192, 576 key tokens) -> concat channels
  -> 3x3 conv + LeakyReLU(0.2).

Sharding: 8 cores = (video b in {0,1}) x (frame t in {0..3}). Each core
computes its full output frame [256,96,96]; K/V are recomputed per core
from its 4-frame video slice (no collectives). Host rotates frames so
xv[0] is the core's own frame; P columns and V tokens both use processed
order, so attention math is order-invariant.

All matmuls are bf16 with fp32 PSUM accumulation. x loads via SWDGE
cast-DMA (f32 DRAM -> bf16 SBUF); a bf16 copy of x is stashed to DRAM
during the K loop and re-read by the two V passes. Window gathers are a
handful of big multi-dim strided copies (Q's gather is folded into the
conv PSUM evacuation). V^T tiles are frame-aligned: br0 = 5 tiles/frame
(last 64 tokens short), br1 = 1 full tile/frame + one shared spill tile
(16 tokens/frame at partition offset f*32). Attention outputs stay in
SBUF (bf16) through the 3x3 conv.
"""

import sys

if "/opt/trn_rl_repo" not in sys.path:
    sys.path.insert(0, "/opt/trn_rl_repo")

import math
from contextlib import ExitStack

import numpy as np

import concourse.bass as bass
import concourse.tile as tile
from concourse import bacc, mybir
from concourse.masks import make_identity

F32 = mybir.dt.float32
F32R = mybir.dt.float32r
BF16 = mybir.dt.bfloat16

T = 4
C = 256
H = W = 96
PIX = H * W
NCORES = 8

PSZ = [4, 8]
OHB = [24, 12]                  # token grid side per branch
NTF = [576, 144]                # real tokens per frame
NTFP = [640, 144]               # P-column stride per frame
NKP = [2560, 640]               # key-token tiles * 128 per video
NQ = [576, 144]                 # query tokens (one frame)
NCH = [16, 64]                  # d-chunks (psz^2)
SC = [1.0 / math.sqrt(2048.0), 1.0 / math.sqrt(8192.0)]
NQB = [[(0, 128), (128, 128), (256, 128), (384, 128), (512, 64)],
       [(0, 128), (128, 16)]]

Exp = mybir.ActivationFunctionType.Exp
Identity = mybir.ActivationFunctionType.Identity


def build(nc):
    xv = nc.dram_tensor("xv", [T, C, PIX], F32R, kind="ExternalInput")
    wqt = nc.dram_tensor("wqt", [C, C], F32R, kind="ExternalInput")
    wkt = nc.dram_tensor("wkt", [C, C], F32R, kind="ExternalInput")
    wvt = nc.dram_tensor("wvt", [C, C], F32R, kind="ExternalInput")
    wot = nc.dram_tensor("wot", [9, C, C], F32R, kind="ExternalInput")
    bq = nc.dram_tensor("bq", [C], F32, kind="ExternalInput")
    bk = nc.dram_tensor("bk", [C], F32, kind="ExternalInput")
    bv = nc.dram_tensor("bv", [C], F32, kind="ExternalInput")
    bo = nc.dram_tensor("bo", [C], F32, kind="ExternalInput")
    out = nc.dram_tensor("out", [C, PIX], F32, kind="ExternalOutput")

    alt = [0]

    def evac_alt(dst, src, bias_ap=None):
        """PSUM -> SBUF evacuation, alternating scalar/vector engines."""
        alt[0] ^= 1
        if bias_ap is not None:
            if alt[0]:
                nc.scalar.activation(out=dst, in_=src, func=Identity,
                                     bias=bias_ap, scale=1.0)
            else:
                nc.vector.tensor_scalar_add(dst, src, bias_ap)
        else:
            if alt[0]:
                nc.scalar.copy(dst, src)
            else:
                nc.vector.tensor_copy(dst, src)

    galt = [0]

    def gather_alt(dst, src):
        galt[0] = (galt[0] + 1) % 3
        if galt[0] == 0:
            nc.vector.tensor_copy(dst, src)
        elif galt[0] == 1:
            nc.scalar.copy(dst, src)
        else:
            nc.gpsimd.tensor_copy(dst, src)

    with tile.TileContext(nc, pool_alloc_mode="queue") as tc, ExitStack() as top:
        persist = top.enter_context(tc.tile_pool(name="persist", bufs=1))
        dramp = top.enter_context(tc.tile_pool(name="dram", bufs=1,
                                               space="DRAM"))

        # bf16 weights: HWDGE f32 load + vector cast (keeps the SWDGE
        # queue free for the first x frame load)
        wq_sb, wk_sb, wv_sb = [None, None], [None, None], [None, None]
        with tc.tile_pool(name="wld", bufs=2) as p_wld:
            for name, dt_, lst in (("wk", wkt, wk_sb), ("wq", wqt, wq_sb),
                                   ("wv", wvt, wv_sb)):
                for cb in range(2):
                    f = p_wld.tile([128, C], F32R, name="wf", tag="wf")
                    nc.sync.dma_start(
                        out=f, in_=dt_.ap()[cb * 128:(cb + 1) * 128, :])
                    t = persist.tile([128, C], BF16, name=f"{name}{cb}",
                                     tag=f"{name}{cb}")
                    nc.vector.tensor_copy(t, f)
                    lst[cb] = t

        def bias_tile(name, dt_):
            t = persist.tile([128, 2], F32, tag=name)
            nc.sync.dma_start(
                out=t, in_=bass.AP(tensor=dt_.ap().tensor, offset=0,
                                   ap=[[1, 128], [128, 2]]))
            return t

        bq_sb = bias_tile("bq", bq)
        bk_sb = bias_tile("bk", bk)
        bv_sb = bias_tile("bv", bv)
        bo_sb = bias_tile("bo", bo)
        ident = persist.tile([128, 128], BF16, name="ident", tag="ident")
        make_identity(nc, ident)
        zrow = persist.tile([128, 98], BF16, name="zrow", tag="zrow")
        nc.vector.memset(zrow, 0.0)

        # token-major bf16 window-gathered x in DRAM (per branch/frame/cb),
        # written during the K loop, read back by the V passes
        xw_d = [[[dramp.tile([128, NCH[br] * NTF[br]], BF16,
                             name=f"xw{br}{j}{cb}", tag=f"xw{br}{j}{cb}")
                  for cb in range(2)] for j in range(T)] for br in range(2)]

        # ---------------- phase A: per-frame QK conv + scores ------------
        esP = ExitStack()
        p_P = esP.enter_context(tc.tile_pool(name="P", bufs=1))
        p_t = [[p_P.tile([128, NKP[b]], BF16, name=f"p{b}_{i}",
                         tag=f"p{b}_{i}")
                for i in range(len(NQB[b]))] for b in range(2)]
        for b in range(2):
            for i in range(len(NQB[b])):
                nc.vector.memset(p_t[b][i][:, :], 0.0)

        esQW = ExitStack()
        p_qw = esQW.enter_context(tc.tile_pool(name="qw", bufs=1))
        qw = [p_qw.tile([128, NCH[b] * NTF[b]], BF16, name=f"qw{b}",
                        tag=f"qw{b}") for b in range(2)]
        p_run = esQW.enter_context(tc.tile_pool(name="run", bufs=1))
        run_mx = [[p_run.tile([128, 1], F32, name=f"mx{b}_{i}",
                              tag=f"mx{b}_{i}")
                   for i in range(len(NQB[b]))] for b in range(2)]
        run_ls = [[p_run.tile([128, 1], F32, name=f"ls{b}_{i}",
                              tag=f"ls{b}_{i}")
                   for i in range(len(NQB[b]))] for b in range(2)]
        p_stat = esQW.enter_context(tc.tile_pool(name="stat", bufs=4))

        ext = [[0 for _ in NQB[b]] for b in range(2)]   # rescale extent

        def conv_win(xcol, w_sb, b_sb, dst, ps_pool):
            """1x1 conv, PSUM evacuated straight into token-major layout
            dst[b][p, ci*ntf + tok] for both branches (+bias).
            xcol(cb, o) returns the x AP slice [128, 384] at pixel col o."""
            # branch0 (coutb 0): one psum region per token row (384 pix)
            d0 = dst[0].rearrange("p (wy wx oh ow) -> p wy wx oh ow",
                                  wy=4, wx=4, oh=24)
            for g in range(12):               # 2 token rows per psum
                ps = ps_pool.tile([128, 1024], F32, name="cps", tag="cps")
                for half in range(2):
                    oh = g * 2 + half
                    for cb in range(2):
                        nc.tensor.matmul(
                            ps[:, half * 512:half * 512 + 384],
                            w_sb[cb][:, 0:128],
                            xcol(cb, oh * 384),
                            start=(cb == 0), stop=(cb == 1))
                for half in range(2):
                    oh = g * 2 + half
                    src = ps[:, half * 512:half * 512 + 384].rearrange(
                        "p (wy ow wx) -> p wy wx ow", wy=4, ow=24)
                    evac_alt(d0[:, :, :, oh], src, b_sb[:, 0:1])
            # branch1 (coutb 1): half a token row (4 of 8 wy) per region
            d1 = dst[1].rearrange("p (wy wx oh ow) -> p wy wx oh ow",
                                  wy=8, wx=8, oh=12)
            for g in range(12):
                ps = ps_pool.tile([128, 1024], F32, name="cps", tag="cps")
                for half in range(2):
                    o = g * 768 + half * 384
                    for cb in range(2):
                        nc.tensor.matmul(
                            ps[:, half * 512:half * 512 + 384],
                            w_sb[cb][:, 128:256],
                            xcol(cb, o),
                            start=(cb == 0), stop=(cb == 1))
                for half in range(2):
                    oh, wyh = divmod(g * 2 + half, 2)
                    src = ps[:, half * 512:half * 512 + 384].rearrange(
                        "p (wy ow wx) -> p wy wx ow", wy=4, ow=12)
                    evac_alt(d1[:, wyh * 4:(wyh + 1) * 4, :, oh], src,
                             b_sb[:, 1:2])

        with tc.tile_pool(name="kx", bufs=1) as p_kx, \
             tc.tile_pool(name="kw", bufs=1) as p_kw, \
             tc.tile_pool(name="st", bufs=2) as p_st, \
             tc.tile_pool(name="kps", bufs=2, space="PSUM") as p_kps, \
             tc.tile_pool(name="sps0", bufs=2, space="PSUM") as p_sps0, \
             tc.tile_pool(name="sps1", bufs=2, space="PSUM") as p_sps1:
            for j in range(T):
                xb = [[None, None], [None, None]]
                for hf in range(2):
                    for cb in range(2):
                        t = p_kx.tile([128, PIX // 2], BF16,
                                      name=f"kx{cb}{hf}", tag=f"kx{cb}{hf}")
                        nc.gpsimd.dma_start(
                            out=t,
                            in_=xv.ap()[j, cb * 128:(cb + 1) * 128,
                                        hf * 4608:(hf + 1) * 4608])
                        xb[cb][hf] = t

                def xcol(cb, o, xb=xb):
                    hf, lo = divmod(o, 4608)
                    return xb[cb][hf][:, lo:lo + 384]

                kw = [p_kw.tile([128, NCH[b] * NTF[b]], BF16,
                                name=f"kw{b}", tag=f"kw{b}")
                      for b in range(2)]
                conv_win(xcol, wk_sb, bk_sb, kw, p_kps)
                if j == 0:
                    conv_win(xcol, wq_sb, bq_sb, qw, p_kps)

                # ---- window-gather x into token-major DRAM stash ----
                # ci-quarter q of branch br = contiguous cols
                # [q*2304, (q+1)*2304) of xw_d[br][j][cb]
                for br in range(2):
                    psz_, ohb_, ntf_ = PSZ[br], OHB[br], NTF[br]
                    for q in range(4):
                        for cb in range(2):
                            st = p_st.tile([128, 2304], BF16,
                                           name=f"st{cb}", tag=f"st{cb}")
                            nwy = psz_ // 4      # wy's per ci-quarter
                            d4 = st.rearrange(
                                "p (wy wx oh ow) -> p wy wx oh ow",
                                wy=nwy, wx=psz_, oh=ohb_)
                            for wyl in range(nwy):
                                wy = q * nwy + wyl
                                # oh rows split by kx half:
                                # image row = oh*psz_+wy < 48 -> half 0
                                ohsplit = (48 - wy + psz_ - 1) // psz_
                                for hf, (o0, o1) in enumerate(
                                        ((0, ohsplit), (ohsplit, ohb_))):
                                    src = xb[cb][hf].rearrange(
                                        "p (oh wy ow wx) -> p wy wx oh ow",
                                        oh=ohb_ // 2, wy=psz_, ow=ohb_)
                                    gather_alt(
                                        d4[:, wyl, :, o0:o1],
                                        src[:, wy, :,
                                            o0 - hf * (ohb_ // 2):
                                            o1 - hf * (ohb_ // 2)])
                            nc.sync.dma_start(
                                out=xw_d[br][j][cb][:,
                                                    q * 2304:(q + 1) * 2304],
                                in_=st)

                # ---- scores for key frame j, both branches ----
                for b in range(2):
                    psz, ohb, ntf = PSZ[b], OHB[b], NTF[b]
                    nmk = 2 if b == 0 else 1
                    mkw = ntf // nmk              # 288 / 144
                    for nqi, (q0, nqsz) in enumerate(NQB[b]):
                        for mkh in range(nmk):
                            ps = (p_sps0 if b == 0 else p_sps1).tile(
                                [128, mkw], F32, name=f"sps{b}",
                                tag=f"sps{b}")
                            for ci in range(NCH[b]):
                                rhs = kw[b][:, ci * ntf + mkh * mkw:
                                            ci * ntf + (mkh + 1) * mkw]
                                lhsT = qw[b][:, ci * ntf + q0:
                                             ci * ntf + q0 + nqsz]
                                nc.tensor.matmul(
                                    ps[:nqsz], lhsT, rhs,
                                    start=(ci == 0),
                                    stop=(ci == NCH[b] - 1))
                            # online softmax over key blocks
                            o = j * NTFP[b] + mkh * mkw
                            pt = p_t[b][nqi]
                            mx, ls = run_mx[b][nqi], run_ls[b][nqi]
                            bm = p_stat.tile([128, 1], F32, name="bm",
                                             tag="bm")
                            nc.vector.reduce_max(out=bm[:nqsz],
                                                 in_=ps[:nqsz, :],
                                                 axis=mybir.AxisListType.X)
                            if j == 0 and mkh == 0:
                                nc.vector.tensor_copy(mx[:nqsz], bm[:nqsz])
                                nmx = p_stat.tile([128, 1], F32, name="nmx",
                                                  tag="nmx")
                                nc.vector.tensor_scalar_mul(
                                    nmx[:nqsz], mx[:nqsz], -SC[b])
                                nc.scalar.activation(
                                    out=pt[:nqsz, o:o + mkw],
                                    in_=ps[:nqsz, :], func=Exp,
                                    bias=nmx[:nqsz], scale=SC[b],
                                    accum_out=ls[:nqsz])
                            else:
                                nmax = p_stat.tile([128, 1], F32,
                                                   name="nmax", tag="nmax")
                                nc.vector.tensor_max(nmax[:nqsz], mx[:nqsz],
                                                     bm[:nqsz])
                                nmx = p_stat.tile([128, 1], F32, name="nmx",
                                                  tag="nmx")
                                nc.vector.tensor_scalar_mul(
                                    nmx[:nqsz], nmax[:nqsz], -SC[b])
                                delta = p_stat.tile([128, 1], F32,
                                                    name="delta",
                                                    tag="delta")
                                nc.scalar.activation(
                                    out=delta[:nqsz], in_=mx[:nqsz],
                                    func=Exp, bias=nmx[:nqsz], scale=SC[b])
                                e = ext[b][nqi]
                                nc.vector.tensor_scalar_mul(
                                    pt[:nqsz, 0:e], pt[:nqsz, 0:e],
                                    delta[:nqsz])
                                pl = p_stat.tile([128, 1], F32, name="pl",
                                                 tag="pl")
                                nc.scalar.activation(
                                    out=pt[:nqsz, o:o + mkw],
                                    in_=ps[:nqsz, :], func=Exp,
                                    bias=nmx[:nqsz], scale=SC[b],
                                    accum_out=pl[:nqsz])
                                nc.vector.scalar_tensor_tensor(
                                    out=ls[:nqsz], in0=ls[:nqsz],
                                    scalar=delta[:nqsz], in1=pl[:nqsz],
                                    op0=mybir.AluOpType.mult,
                                    op1=mybir.AluOpType.add)
                                nc.vector.tensor_copy(mx[:nqsz],
                                                      nmax[:nqsz])
                            ext[b][nqi] = max(ext[b][nqi], o + mkw)

        # final normalization of P
        for b in range(2):
            for nqi, (q0, nqsz) in enumerate(NQB[b]):
                rs = p_stat.tile([128, 1], F32, name="rs", tag="rs")
                nc.vector.reciprocal(rs[:nqsz], run_ls[b][nqi][:nqsz])
                nc.vector.tensor_scalar_mul(
                    p_t[b][nqi][:nqsz, :], p_t[b][nqi][:nqsz, :], rs[:nqsz])
        esQW.close()

        # pool regions are reserved for a pool's whole open-close span, so
        # the long-lived att/PT pools open after the frame loop's pools
        # close, on the RIGHT stack (LIFO is per (space, side))
        esAtt = ExitStack()
        p_att = esAtt.enter_context(tc.tile_pool(name="att", bufs=1,
                                                 side="right"))
        esPT1 = ExitStack()
        p_PT1 = esPT1.enter_context(tc.tile_pool(name="PT1", bufs=1,
                                                 side="right"))
        esPT0 = ExitStack()
        p_PT0 = esPT0.enter_context(tc.tile_pool(name="PT0", bufs=1,
                                                 side="right"))

        # ---------------- P^T transposes for both branches ----------------
        pt1_t = [p_PT1.tile([128, NQ[1]], BF16, name=f"pt1_{i}",
                            tag=f"pt1_{i}") for i in range(5)]
        nc.gpsimd.memset(pt1_t[4][:, :], 0.0)
        pt0_t = [p_PT0.tile([128, NQ[0]], BF16, name=f"pt0_{i}",
                            tag=f"pt0_{i}") for i in range(NKP[0] // 128)]

        with tc.tile_pool(name="ptps", bufs=4, space="PSUM") as p_ptps:
            for ti in range(NKP[0] // 128):
                for nqi, (q0, nqsz) in enumerate(NQB[0]):
                    tp = p_ptps.tile([128, 128], BF16, name="ptps",
                                     tag="ptps")
                    nc.tensor.transpose(
                        tp[:, :nqsz],
                        p_t[0][nqi][:nqsz, ti * 128:(ti + 1) * 128],
                        ident[:nqsz, :nqsz])
                    evac_alt(pt0_t[ti][:, q0:q0 + nqsz], tp[:, :nqsz])
            for f in range(T):
                for nqi, (q0, nqsz) in enumerate(NQB[1]):
                    tp = p_ptps.tile([128, 128], BF16, name="ptps",
                                     tag="ptps")
                    nc.tensor.transpose(
                        tp[:, :nqsz],
                        p_t[1][nqi][:nqsz, f * 144:f * 144 + 128],
                        ident[:nqsz, :nqsz])
                    evac_alt(pt1_t[f][:, q0:q0 + nqsz], tp[:, :nqsz])
                    tp2 = p_ptps.tile([128, 128], BF16, name="ptps2",
                                      tag="ptps2")
                    nc.tensor.transpose(
                        tp2[:16, :nqsz],
                        p_t[1][nqi][:nqsz, f * 144 + 128:(f + 1) * 144],
                        ident[:nqsz, :nqsz])
                    evac_alt(pt1_t[4][f * 32:f * 32 + 16, q0:q0 + nqsz],
                             tp2[:16, :nqsz])
        esP.close()

        # ---------------- att tiles (SBUF-resident, bf16) -----------------
        att_sb = {}

        def init_att(br):
            att = p_att.tile([128, 98 * 98], BF16, name=f"att{br}",
                             tag=f"att{br}")
            att_sb[br] = att
            attv = att.rearrange("p (h w) -> p h w", h=98)
            nc.gpsimd.tensor_copy(att[:, 0:98], zrow)
            nc.gpsimd.tensor_copy(att[:, 97 * 98:98 * 98], zrow)
            zcol = zrow[:, 0:96].rearrange("p (a c) -> p a c", a=96)
            nc.gpsimd.tensor_copy(attv[:, 1:97, 0:1], zcol)
            nc.gpsimd.tensor_copy(attv[:, 1:97, 97:98], zcol)
            return attv

        # ---------------- phase C: V build + PV, per branch ----------------
        for br in range(2):
            psz, ohb, ntf = PSZ[br], OHB[br], NTF[br]
            ntiles = NKP[br] // 128
            esV = ExitStack()
            p_V = esV.enter_context(tc.tile_pool(name=f"V{br}", bufs=1))
            v_t = [p_V.tile([128, NCH[br] * 128], BF16, name=f"v{br}_{i}",
                            tag=f"v{br}_{i}") for i in range(ntiles)]
            if br == 0:
                for f in range(T):
                    nc.gpsimd.memset(v_t[5 * f + 4][64:128, :], 0.0)
            else:
                nc.gpsimd.memset(v_t[4][:, :], 0.0)

            with tc.tile_pool(name=f"xq{br}", bufs=2) as p_xq, \
                 tc.tile_pool(name=f"vps{br}", bufs=2 if br else 4,
                              space="PSUM") as p_vps:
                for j in range(T):
                    for q in range(4):      # ci-quarter, cols [q*2304, ..)
                        xq = []
                        for cb in range(2):
                            t = p_xq.tile([128, 2304], BF16,
                                          name=f"xq{cb}", tag=f"xq{cb}")
                            nc.sync.dma_start(
                                out=t,
                                in_=xw_d[br][j][cb][:,
                                                    q * 2304:(q + 1) * 2304])
                            xq.append(t)
                        if br == 0:
                            for sub in range(5):
                                m = 128 if sub < 4 else 64
                                t0 = sub * 128
                                ti = 5 * j + sub
                                ps = p_vps.tile([128, 512], F32,
                                                name="vps", tag="vps")
                                for cl in range(4):
                                    for cb in range(2):
                                        lhsT = xq[cb][:, cl * ntf + t0:
                                                      cl * ntf + t0 + m]
                                        nc.tensor.matmul(
                                            ps[:m, cl * 128:(cl + 1) * 128],
                                            lhsT,
                                            wv_sb[cb][:, 0:128],
                                            start=(cb == 0),
                                            stop=(cb == 1))
                                evac_alt(v_t[ti][:m,
                                                 q * 512:(q + 1) * 512],
                                         ps[:m, :])
                        else:
                            ps = p_vps.tile([128, 2048], F32, name="vps",
                                            tag="vps")
                            for cl in range(16):
                                for cb in range(2):
                                    lhsT = xq[cb][:, cl * ntf:
                                                  cl * ntf + 128]
                                    nc.tensor.matmul(
                                        ps[:, cl * 128:(cl + 1) * 128],
                                        lhsT,
                                        wv_sb[cb][:, 128:256],
                                        start=(cb == 0),
                                        stop=(cb == 1))
                            evac_alt(v_t[j][:, q * 2048:(q + 1) * 2048],
                                     ps)
                            off = j * 32
                            ps2 = p_vps.tile([128, 2048], F32, name="vps",
                                             tag="vps")
                            for cl in range(16):
                                for cb in range(2):
                                    lhsT = xq[cb][:, cl * ntf + 128:
                                                  cl * ntf + 144]
                                    nc.tensor.matmul(
                                        ps2[off:off + 16,
                                            cl * 128:(cl + 1) * 128],
                                        lhsT,
                                        wv_sb[cb][:, 128:256],
                                        start=(cb == 0),
                                        stop=(cb == 1),
                                        tile_position=(0, off))
                            evac_alt(
                                v_t[4][off:off + 16,
                                       q * 2048:(q + 1) * 2048],
                                ps2[off:off + 16, :])

            # --- PV: y^T accumulated over all key tiles; write into att ---
            attv = init_att(br)
            wvw = attv[:, 1:97, 1:97].rearrange(
                "p (oh hh) (ow ww) -> p oh hh ow ww", hh=psz, ww=psz)
            pt_t = pt0_t if br == 0 else pt1_t
            nqh_n = 2 if br == 0 else 1
            nqw = NQ[br] // nqh_n
            ohq = ohb // nqh_n
            with tc.tile_pool(name=f"pvps{br}", bufs=4,
                              space="PSUM") as p_pvps:
                for ci in range(NCH[br]):
                    wy, wx = divmod(ci, psz)
                    for nqh in range(nqh_n):
                        ps = p_pvps.tile([128, nqw], F32, name="pvps",
                                         tag="pvps")
                        for ti in range(ntiles):
                            nc.tensor.matmul(
                                ps, v_t[ti][:, ci * 128:(ci + 1) * 128],
                                pt_t[ti][:, nqh * nqw:(nqh + 1) * nqw],
                                start=(ti == 0), stop=(ti == ntiles - 1))
                        dst = wvw[:, nqh * ohq:(nqh + 1) * ohq, wy, :, wx]
                        src = ps.rearrange("p (a c) -> p a c", a=ohq)
                        evac_alt(dst, src, bv_sb[:, br:br + 1])
            esV.close()
            if br == 0:
                esPT0.close()
        esPT1.close()

        # ---------------- phase D: 3x3 conv + LeakyReLU ----------------
        with tc.tile_pool(name="wot", bufs=1) as p_wot, \
             tc.tile_pool(name="dout", bufs=3) as p_do, \
             tc.tile_pool(name="dps", bufs=2, space="PSUM") as p_dps:
            wot_sb = []
            for cb in range(2):
                t = p_wot.tile([128, 9, C], BF16, name=f"wot{cb}",
                               tag=f"wot{cb}")
                nc.gpsimd.dma_start(
                    out=t,
                    in_=wot.ap()[:, cb * 128:(cb + 1) * 128, :].rearrange(
                        "t i o -> i t o"))
                wot_sb.append(t)
            attv2 = [att_sb[cb].rearrange("p (h w) -> p h w", h=98)
                     for cb in range(2)]
            # 20 row-groups: 16x5 rows + 4x4 rows, in groups of 4
            RG = [(i * 5, 5) for i in range(16)] + \
                 [(80 + i * 4, 4) for i in range(4)]
            for coutb in range(2):
                for g in range(5):
                    grp = RG[g * 4:(g + 1) * 4]
                    nr = grp[0][1]
                    w = nr * 96
                    ps = p_dps.tile([128, 2048], F32, name="dps",
                                    tag="dps")
                    for cb in range(2):
                        for tap in range(9):
                            dy, dx = divmod(tap, 3)
                            lhsT = wot_sb[cb][:, tap,
                                              coutb * 128:(coutb + 1) * 128]
                            for rg_, (r0, _) in enumerate(grp):
                                rhs = attv2[cb][:, r0 + dy:r0 + dy + nr,
                                                dx:dx + 96]
                                nc.tensor.matmul(
                                    ps[:, rg_ * 512:rg_ * 512 + w],
                                    lhsT, rhs,
                                    start=(cb == 0 and tap == 0),
                                    stop=(cb == 1 and tap == 8))
                    psv = ps.rearrange("p (a c) -> p a c", a=4)[:, :, 0:w]
                    t1 = p_do.tile([128, 4 * 480], F32, name="t1",
                                   tag="t1")
                    t1v = t1[:, 0:4 * w].rearrange("p (a c) -> p a c", a=4)
                    nc.scalar.activation(out=t1v, in_=psv, func=Identity,
                                         bias=bo_sb[:, coutb:coutb + 1],
                                         scale=1.0)
                    t2 = p_do.tile([128, 4 * 480], F32, name="t2",
                                   tag="t2")
                    nc.vector.scalar_tensor_tensor(
                        out=t2[:, 0:4 * w], in0=t1[:, 0:4 * w], scalar=0.2,
                        in1=t1[:, 0:4 * w],
                        op0=mybir.AluOpType.mult,
                        op1=mybir.AluOpType.max)
                    nc.sync.dma_start(
                        out=out.ap()[coutb * 128:(coutb + 1) * 128,
                                     grp[0][0] * 96:
                                     grp[0][0] * 96 + 4 * w],
                        in_=t2[:, 0:4 * w])
        esAtt.close()
    return nc


_CACHED = {}


def _get_nc():
    if "nc" not in _CACHED:
        nc = bacc.Bacc("TRN2", debug=False, target_bir_lowering=False)
        build(nc)
        nc.compile()
        _CACHED["nc"] = nc
    return _CACHED["nc"]


def make_in_maps(x, wq, bq_, wk, bk_, wv, bv_, wo, bo_):
    shared = {
        "wqt": np.ascontiguousarray(wq.T.astype(np.float32)),
        "wkt": np.ascontiguousarray(wk.T.astype(np.float32)),
        "wvt": np.ascontiguousarray(wv.T.astype(np.float32)),
        "wot": np.ascontiguousarray(
            wo.transpose(2, 3, 1, 0).reshape(9, C, C).astype(np.float32)),
        "bq": np.ascontiguousarray(bq_.astype(np.float32)),
        "bk": np.ascontiguousarray(bk_.astype(np.float32)),
        "bv": np.ascontiguousarray(bv_.astype(np.float32)),
        "bo": np.ascontiguousarray(bo_.astype(np.float32)),
    }
    x3 = np.ascontiguousarray(x.reshape(2 * T, C, PIX).astype(np.float32))
    in_maps = []
    for core in range(NCORES):
        v, f = divmod(core, T)
        # rotate so the core's own frame is xv[0]; P and V both use
        # processed order, so attention math is order-invariant.
        order = [f] + [k for k in range(T) if k != f]
        m = dict(shared)
        m["xv"] = np.ascontiguousarray(x3[[v * T + k for k in order]])
        in_maps.append(m)
    return in_maps


def kernel(**inputs):
    from concourse.bass_utils import run_bass_kernel_spmd

    x = np.asarray(inputs["x"], dtype=np.float32)
    in_maps = make_in_maps(
        x, np.asarray(inputs["wq"]), np.asarray(inputs["bq"]),
        np.asarray(inputs["wk"]), np.asarray(inputs["bk"]),
        np.asarray(inputs["wv"]), np.asarray(inputs["bv"]),
        np.asarray(inputs["wo"]), np.asarray(inputs["bo"]))
    nc = _get_nc()
    res = run_bass_kernel_spmd(nc, in_maps, core_ids=list(range(NCORES)))
    outs = [res.results[c]["out"].reshape(C, H, W) for c in range(NCORES)]
    return np.stack(outs).astype(np.float32)
